# revision 35
# baseline (speedup 1.0000x reference)
"""Multi-head attention with fraction-based RoPE ("stoich RoPE") on 8
Trainium2 NeuronCores.

Sharding: each core owns one (batch, query-half) pair — B=4 batches x 2
query halves = 8 shards.  Every core projects Q for its 1024 query rows
and K/V for the full 2048 keys of its batch (K/V projection is computed
on both cores sharing a batch; the 2x redundancy buys a kernel with no
collectives: the attention output rows owned by a core carry the full
head dimension, so the output projection and bias are entirely local).

Per-core device program (SPMD, identical on all 8 cores):
  phase A  per head-pair (8 x 128 dims): project Q^T/K^T/V^T from x^T
           streamed out of DRAM (weights stationary, x moving), add
           biases, apply RoPE to Q/K via precomputed cos/sin tiles and
           32-partition cross-quadrant swaps, PE-transpose V into
           natural layout with a ones column appended (row 64 of the
           P@V' output then carries the softmax denominator).
  phase B  attention per head-pair: both heads' scores are issued
           back-to-back as 64x128 PE row-tiles (tile positions (0,0) /
           (64,0)) so they execute concurrently -> one exp per chunk on
           ACT over both heads' banks (scale=1/8 folded in, no max
           subtraction: |scores/8| is O(1) for this operator's input
           distribution) -> two P^T@V' accumulators (head 1's V tile is
           column-shifted so its output lands on partitions 64..127) ->
           denominator reciprocal via approx-fast DVE op + GPSIMD
           partition_broadcast + one DVE multiply; the normalize chain
           touches no PE instruction, so the PE never stalls on it.
  phase C  output projection: attn^T chunks stationary, Wo^T moving,
           + bias, DMA out rows.

The host shards/formats inputs (transposes, bias/cos-sin tiles) and
concatenates the 8 output row-shards.
"""

import contextlib
import ctypes
import sys
import types

import numpy as np
import ml_dtypes

import concourse.bass as bass
import concourse.mybir as mybir
import concourse.tile as tile
from concourse.masks import make_identity
from concourse.vector_clock import ScopedClock

# ---------------- problem constants (hardcoded per contract) ----------------
B, T, D = 4, 2048, 1024
H, HD = 16, 64  # heads, head dim
HALF = HD // 2
N_CORES = 8
TQ = T // 2  # query rows per core
P = 128
NQ = 512  # moving-dim tile for matmuls
NPAIR = D // P  # 8 head pairs per core
SCALE = 1.0 / np.sqrt(HD)  # folded into exp()
ROPE_SCALE = 1000.0
ROPE_BASE = 10000.0

F32 = mybir.dt.float32
DT_MM = mybir.dt.bfloat16  # dtype of matmul operands (bfloat16 | float32)

_SO_PATH = "/opt/axon/libaxon_pjrt.so"


# ---------------- axon/NTFF environment shims ----------------
def _ntff_profile_hook():
    try:
        lib = ctypes.CDLL(_SO_PATH)
    except OSError:
        return None
    if not hasattr(lib, "axon_start_nrt_profile"):
        return None
    lib.axon_start_nrt_profile.argtypes = [
        ctypes.POINTER(ctypes.c_int64),
        ctypes.c_size_t,
    ]
    lib.axon_start_nrt_profile.restype = ctypes.c_int64
    lib.axon_stop_nrt_profile.argtypes = [ctypes.c_char_p]
    lib.axon_stop_nrt_profile.restype = ctypes.c_int64

    @contextlib.contextmanager
    def _hook(output_dir, device_ids):
        import jax

        jax.devices()
        if device_ids:
            ids = (ctypes.c_int64 * len(device_ids))(*device_ids)
            rc = lib.axon_start_nrt_profile(ids, len(device_ids))
        else:
            rc = lib.axon_start_nrt_profile(None, 0)
        if rc != 0:
            raise RuntimeError(f"axon_start_nrt_profile rc={rc}")
        try:
            yield
        finally:
            n = lib.axon_stop_nrt_profile(str(output_dir).encode())
            if n < 0:
                raise RuntimeError(f"axon_stop_nrt_profile rc={n}")

    return _hook


def install_shims():
    if "antenv.axon_hooks" not in sys.modules:
        mod = types.ModuleType("antenv.axon_hooks")
        hook = _ntff_profile_hook()
        mod.get_axon_ntff_profile_hook = lambda: hook
        mod.set_axon_ntff_profile_hook = lambda h: None
        sys.modules["antenv.axon_hooks"] = mod
    import concourse.bass_utils as bass_utils

    bass_utils.upload_artifacts = lambda tmpdir: str(tmpdir)

    import os

    if os.environ.get("BASS_LDW_OPT") == "1" and not getattr(
        bass_utils, "_ldw_opt_patched", False
    ):
        orig_run = bass_utils.run_command

        def _run_ldw(argv, **kw):
            argv = [
                "--enable-ldw-opt=true" if a == "--enable-ldw-opt=false" else a
                for a in argv
            ]
            return orig_run(argv, **kw)

        bass_utils.run_command = _run_ldw
        bass_utils._ldw_opt_patched = True


class TileContextSplitDrain(tile.TileContext):
    """This walrus build encodes at most 2 sync waits per CTRL
    instruction; Tile's kernel-tail drain wants one wait per logical
    processor.  Split the waits across single-wait NOPs instead."""

    MAX_WAITS = 1

    def _drain_and_barrier(self, tick_clock, wait_clock):
        nc = self.nc
        carrier = nc.sync.nop(nofuse=True)
        wait_clock.add_sem_waits(
            carrier.ins, ScopedClock({None: tick_clock.global_clock})
        )
        waits = list(carrier.ins.sync_info.on_wait or [])
        if len(waits) > self.MAX_WAITS:
            carrier.ins.sync_info.on_wait[:] = waits[: self.MAX_WAITS]
            for i in range(self.MAX_WAITS, len(waits), self.MAX_WAITS):
                extra = nc.sync.nop(nofuse=True)
                extra.ins.sync_info = mybir.SyncInfo(
                    on_wait=list(waits[i : i + self.MAX_WAITS]), on_update=[]
                )
        nc.sync.drain()
        nc.all_engine_barrier()
        assert self.sems is not None
        popped = nc._tile_sem_poison_stack.pop()
        assert popped is self._sem_poison
        nc.clear_and_free_semaphores(list(self.sems.allocated().values()))
        nc.all_engine_barrier()


def _split_sync_waits(nc, max_waits=1):
    """This walrus build rejects instructions carrying more than ~2 sync
    waits.  Move excess waits onto same-engine NOPs inserted just before
    the instruction (AND semantics are preserved: the engine blocks on
    each carrier in program order)."""
    for f in nc.m.functions:
        for bb in f.blocks:
            out = []
            for inst in bb.instructions:
                si = inst.sync_info
                waits = list(si.on_wait) if si and si.on_wait else []
                if len(waits) > max_waits:
                    for i in range(0, len(waits) - max_waits, max_waits):
                        nop = mybir.InstNoOp(
                            name=nc.get_next_instruction_name(), ins=[], outs=[]
                        )
                        nop.engine = inst.engine
                        nop.sync_info = mybir.SyncInfo(
                            on_wait=list(waits[i : i + max_waits]), on_update=[]
                        )
                        nc.register_instruction(nop, overwrite=True)
                        out.append(nop)
                    si.on_wait[:] = waits[len(waits) - max_waits :]
                out.append(inst)
            bb.instructions[:] = out


# ---------------- device program ----------------
def build_nc(dt_mm=DT_MM):
    nc = bass.Bass(
        "TRN2", target_bir_lowering=False, debug=False, num_devices=N_CORES
    )

    xt = nc.dram_tensor("xt", [D, T], dt_mm, kind="ExternalInput")
    xtq = nc.dram_tensor("xtq", [D, TQ], dt_mm, kind="ExternalInput")
    wqt = nc.dram_tensor("wqt", [D, D], dt_mm, kind="ExternalInput")
    wkt = nc.dram_tensor("wkt", [D, D], dt_mm, kind="ExternalInput")
    wvt = nc.dram_tensor("wvt", [D, D], dt_mm, kind="ExternalInput")
    wot = nc.dram_tensor("wot", [D, D], dt_mm, kind="ExternalInput")
    bq = nc.dram_tensor("bq", [P, NPAIR], F32, kind="ExternalInput")
    bk = nc.dram_tensor("bk", [P, NPAIR], F32, kind="ExternalInput")
    bv = nc.dram_tensor("bv", [P, NPAIR], F32, kind="ExternalInput")
    bob = nc.dram_tensor("bob", [P, D], F32, kind="ExternalInput")
    csaq = nc.dram_tensor("csaq", [P, TQ], dt_mm, kind="ExternalInput")
    csbq = nc.dram_tensor("csbq", [P, TQ], dt_mm, kind="ExternalInput")
    csak = nc.dram_tensor("csak", [P, T], dt_mm, kind="ExternalInput")
    csbk = nc.dram_tensor("csbk", [P, T], dt_mm, kind="ExternalInput")
    out = nc.dram_tensor("out", [TQ, D], F32, kind="ExternalOutput")

    with TileContextSplitDrain(nc) as tc:
        persist_cm = tc.tile_pool(name="persist", bufs=1)
        persist = persist_cm.__enter__()

        def ptile(shape, dt, tag):
            return persist.tile(shape, dt, tag=tag, name=tag)

        with contextlib.ExitStack() as ctx:
            # ---- persistent tiles ----
            csaq_t = ptile([P, TQ], dt_mm, "csaq_t")
            csbq_t = ptile([P, TQ], dt_mm, "csbq_t")
            csak_t = ptile([P, T], dt_mm, "csak_t")
            csbk_t = ptile([P, T], dt_mm, "csbk_t")
            bq_t = ptile([P, NPAIR], F32, "bq_t")
            bk_t = ptile([P, NPAIR], F32, "bk_t")
            bv_t = ptile([P, NPAIR], F32, "bv_t")
            ident = ptile([P, HD], dt_mm, "ident")
            attn = [ptile([P, TQ], dt_mm, f"attn{pr}") for pr in range(NPAIR)]
            # out-projection weights prefetched up front (DMA engines are
            # otherwise idle mid-kernel; avoids a phase-C load stall)
            bob_t = ptile([P, D], F32, "bob_t")
            wo_c = [ptile([P, D], dt_mm, f"wo{ch}") for ch in range(NPAIR)]
            ones64 = ptile([1, HD], mybir.dt.float32r, "ones64")
            ones64_f = ptile([1, HD], F32, "ones64_f")
            nc.sync.dma_start(csaq_t[:], csaq[:])
            nc.sync.dma_start(csbq_t[:], csbq[:])
            nc.sync.dma_start(csak_t[:], csak[:])
            nc.sync.dma_start(csbk_t[:], csbk[:])
            nc.sync.dma_start(bq_t[:], bq[:])
            nc.sync.dma_start(bk_t[:], bk[:])
            nc.sync.dma_start(bv_t[:], bv[:])
            nc.sync.dma_start(bob_t[:], bob[:])
            for ch in range(NPAIR):
                nc.sync.dma_start(wo_c[ch][:], wot[ch * P : (ch + 1) * P, :])
            make_identity(nc, ident[0:HD, :])
            make_identity(nc, ident[HD : 2 * HD, :])
            nc.vector.memset(ones64_f[:], 1.0)
            with nc.allow_low_precision(reason="ones vector for f32r bcast"):
                nc.scalar.copy(ones64[:], ones64_f[:])

            # ---- pools for the head-pair loop ----
            big = 2 if dt_mm != F32 else 1
            xp = ctx.enter_context(tc.tile_pool(name="xp", bufs=3))
            wp = ctx.enter_context(tc.tile_pool(name="wp", bufs=2))
            rawp = ctx.enter_context(tc.tile_pool(name="rawp", bufs=2))
            ropep = ctx.enter_context(tc.tile_pool(name="ropep", bufs=1))
            vtp = ctx.enter_context(tc.tile_pool(name="vtp", bufs=1))
            qkp = ctx.enter_context(tc.tile_pool(name="qkp", bufs=big))
            vnp = ctx.enter_context(tc.tile_pool(name="vnp", bufs=big))
            exp_p = ctx.enter_context(tc.tile_pool(name="exp_p", bufs=4))
            smallp = ctx.enter_context(tc.tile_pool(name="smallp", bufs=4))
            sumsp = ctx.enter_context(tc.tile_pool(name="sumsp", bufs=2))
            recp = ctx.enter_context(tc.tile_pool(name="recp", bufs=2))
            tailp = ctx.enter_context(tc.tile_pool(name="tailp", bufs=2))
            ps_proj = ctx.enter_context(
                tc.tile_pool(name="ps_proj", bufs=2, space="PSUM")
            )
            ps_sc = ctx.enter_context(
                tc.tile_pool(name="ps_sc", bufs=2, space="PSUM")
            )
            ps_po = ctx.enter_context(
                tc.tile_pool(name="ps_po", bufs=1, space="PSUM")
            )

            def rope(raw, ntok, csa_t, csb_t, out_tile):
                # raw/cs/out all dt_mm [P, ntok]
                m1 = ropep.tile([P, T], dt_mm, tag="m1", name="m1")
                m2 = ropep.tile([P, T], dt_mm, tag="m2", name="m2")
                t32 = ropep.tile([32, T], dt_mm, tag="t32", name="t32")
                nc.vector.tensor_mul(m1[:, :ntok], raw[:], csa_t[:, :ntok])
                nc.vector.tensor_mul(m2[:, :ntok], raw[:], csb_t[:, :ntok])
                # swap 32-halves within each 64-block of m2 (in place via t32)
                for blk in range(2):
                    b0 = blk * 64
                    nc.vector.tensor_copy(t32[:, :ntok], m2[b0 : b0 + 32, :ntok])
                    nc.vector.tensor_copy(
                        m2[b0 : b0 + 32, :ntok], m2[b0 + 32 : b0 + 64, :ntok]
                    )
                    nc.vector.tensor_copy(
                        m2[b0 + 32 : b0 + 64, :ntok], t32[:, :ntok]
                    )
                nc.vector.tensor_add(out_tile[:], m1[:, :ntok], m2[:, :ntok])

            def stage_units(pr):
                """Emission units for pair pr's projections + RoPE + V
                transpose.  Each unit emits a small instruction group; the
                attention loop of the previous pair pumps these so the PE
                stays dense while ACT works on exp."""
                d0 = pr * P
                st = {}
                units = []

                def u_wdma():
                    st["wq"] = wp.tile([P, NPAIR, P], dt_mm, tag="wq", name="wq_c")
                    st["wk"] = wp.tile([P, NPAIR, P], dt_mm, tag="wk", name="wk_c")
                    st["wv"] = wp.tile([P, NPAIR, P], dt_mm, tag="wv", name="wv_c")
                    for key, w in (("wq", wqt), ("wk", wkt), ("wv", wvt)):
                        nc.sync.dma_start(
                            st[key][:],
                            w[:, d0 : d0 + P].rearrange("(f p) d -> p f d", p=P),
                        )
                    st["qraw"] = rawp.tile([P, TQ], dt_mm, tag="qraw", name="q_raw")
                    st["kraw"] = rawp.tile([P, T], dt_mm, tag="kraw", name="k_raw")
                    st["vt"] = vtp.tile([P, T], dt_mm, tag="vt", name="v_t")

                units.append(u_wdma)

                def u_xdma(key, nb, src):
                    def go():
                        xc = xp.tile([P, NPAIR, NQ], dt_mm, tag="xc", name="xc")
                        nc.sync.dma_start(
                            xc[:],
                            src[:, nb * NQ : (nb + 1) * NQ].rearrange(
                                "(f p) t -> p f t", p=P
                            ),
                        )
                        st[key] = xc

                    return go

                def u_mm(w_key, x_key, f, start, stop):
                    def go():
                        if start:
                            st["ps"] = ps_proj.tile(
                                [P, NQ], F32, tag="ps", name="ps"
                            )
                        nc.tensor.matmul(
                            st["ps"][:],
                            st[w_key][:, f, :],
                            st[x_key][:, f, :],
                            start=start,
                            stop=stop,
                        )

                    return go

                def u_evict(b_t, dst_key, dslice):
                    # bias-add eviction on DVE (per-partition scalar
                    # operand) keeps ACT free for the exp stream
                    def go():
                        nc.vector.tensor_scalar_add(
                            st[dst_key][:, dslice],
                            st["ps"][:],
                            b_t[:, pr : pr + 1],
                        )

                    return go

                def u_rope():
                    st["qt"] = qkp.tile([P, TQ], dt_mm, tag="qt", name="qt")
                    rope(st["qraw"], TQ, csaq_t, csbq_t, st["qt"])

                def u_rope2():
                    st["kt"] = qkp.tile([P, T], dt_mm, tag="kt", name="kt")
                    rope(st["kraw"], T, csak_t, csbk_t, st["kt"])

                # all DMAs first: deep prefetch so pumped matmuls never
                # wait on HBM
                for nb in range(T // NQ):
                    units.append(u_xdma("x%d" % nb, nb, xt))
                for nb in range(TQ // NQ):
                    units.append(u_xdma("q%d" % nb, nb, xtq))
                for nb in range(T // NQ):
                    for w_key, b_t, dst_key in (("wk", bk_t, "kraw"), ("wv", bv_t, "vt")):
                        for f in range(NPAIR):
                            units.append(
                                u_mm(w_key, "x%d" % nb, f, f == 0, f == NPAIR - 1)
                            )
                        units.append(
                            u_evict(b_t, dst_key, slice(nb * NQ, (nb + 1) * NQ))
                        )
                        # rope K as soon as its last block is evicted so the
                        # next pair's first scores never wait on it
                        if nb == T // NQ - 1 and dst_key == "kraw":
                            units.append(u_rope2)
                for nb in range(TQ // NQ):
                    for f in range(NPAIR):
                        units.append(u_mm("wq", "q%d" % nb, f, f == 0, f == NPAIR - 1))
                    units.append(
                        u_evict(bq_t, "qraw", slice(nb * NQ, (nb + 1) * NQ))
                    )
                units.append(u_rope)

                def u_vn_alloc(hh):
                    # head 0: V dims in cols 0..63, ones (denominator) in
                    # col 64 -> P@V' rows 0..63 attn / row 64 denom.
                    # head 1: ones in col 0, V dims in cols 64..127 ->
                    # row 0 denom / rows 64..127 attn, so the pair's
                    # normalized output assembles with zero cross-
                    # partition copies.
                    def go():
                        vn_h = vnp.tile(
                            [P, T // P, P], dt_mm, tag=f"vn{hh}", name="vn_h"
                        )
                        if hh == 0:
                            nc.vector.memset(vn_h[:, :, HD : HD + 1], 1.0)
                            nc.vector.memset(vn_h[:, :, HD + 1 :], 0.0)
                        else:
                            nc.vector.memset(vn_h[:, :, 0:1], 1.0)
                            nc.vector.memset(vn_h[:, :, 1:HD], 0.0)
                        st[f"vn{hh}"] = vn_h

                    return go

                def u_vtr(hh, ch):
                    def go():
                        tp = ps_proj.tile([P, HD], dt_mm, tag="ps", name="tp")
                        h0 = hh * HD
                        nc.tensor.transpose(
                            tp[:],
                            st["vt"][h0 : h0 + HD, ch * P : (ch + 1) * P],
                            ident[h0 : h0 + HD, :],
                        )
                        dcols = slice(0, HD) if hh == 0 else slice(HD, 2 * HD)
                        nc.vector.tensor_copy(st[f"vn{hh}"][:, ch, dcols], tp[:])

                    return go

                # allocate both V tiles first, then interleave the two
                # heads' transposes per chunk: adjacent 64-row transposes
                # can tile-overlap, and early chunks of BOTH heads are
                # ready before the next pair's first PV needs them
                units.append(u_vn_alloc(0))
                units.append(u_vn_alloc(1))
                for ch in range(T // P):
                    units.append(u_vtr(0, ch))
                    units.append(u_vtr(1, ch))
                return st, units

            def pump(units, n):
                for _ in range(n):
                    if units:
                        units.pop(0)()

            def make_norm(pr, qs, au, sums):
                """Deferred normalize for one q-block: broadcast the raw
                denominators across partitions with two concurrent col-tiled
                K=1 matmuls (they wait only on fast DVE copies), then one
                128-lane DVE reciprocal and one multiply.  Emitted a couple
                of chunks into the NEXT q-block so the PE queue never waits
                on the DVE."""

                def go():
                    # matmul dst must start at partition 0 on this toolchain,
                    # so broadcast each head's denominators into its own
                    # base-0 PSUM tile and assemble with cross-offset copies
                    pb_a = ps_proj.tile([P, NQ], F32, tag="ps", name="pb_a")
                    pb_b = ps_proj.tile([P, NQ], F32, tag="ps", name="pb_b")
                    nc.tensor.matmul(
                        pb_a[0:HD, :], ones64[:], sums[:, 0:NQ],
                        start=True, stop=True,
                    )
                    nc.tensor.matmul(
                        pb_b[0:HD, :], ones64[:], sums[:, NQ : 2 * NQ],
                        start=True, stop=True,
                    )
                    rb = recp.tile([P, NQ], F32, tag="rb", name="rb")
                    nc.vector.tensor_copy(rb[0:HD, :], pb_a[0:HD, :])
                    nc.vector.tensor_copy(rb[HD:, :], pb_b[0:HD, :])
                    recb = recp.tile([P, NQ], F32, tag="recb", name="recb")
                    nc.vector.reciprocal(recb[:], rb[:])
                    nc.vector.tensor_mul(attn[pr][:, qs], au[:], recb[:])

                return go

            def make_tail_units():
                """Out-projection units for the first four q-row blocks
                (columns owned by q-block 0).  Pumped into pair 7's
                attention loop, whose pump stream is otherwise empty."""
                units = []
                for tb in range(TQ // P // 2):
                    ts = slice(tb * P, (tb + 1) * P)
                    for nh in range(2):
                        stt = {}

                        def ua(stt=stt, ts=ts, nh=nh):
                            stt["pout"] = ps_proj.tile(
                                [P, NQ], F32, tag="ps", name="pout_t"
                            )
                            for ch in range(4):
                                nc.tensor.matmul(
                                    stt["pout"][:],
                                    attn[ch][:, ts],
                                    wo_c[ch][:, nh * NQ : (nh + 1) * NQ],
                                    start=(ch == 0),
                                    stop=False,
                                )

                        def ub(stt=stt, ts=ts, nh=nh):
                            for ch in range(4, NPAIR):
                                nc.tensor.matmul(
                                    stt["pout"][:],
                                    attn[ch][:, ts],
                                    wo_c[ch][:, nh * NQ : (nh + 1) * NQ],
                                    start=False,
                                    stop=(ch == NPAIR - 1),
                                )
                            osb = tailp.tile(
                                [P, NQ], F32, tag="osb", name="osb_t"
                            )
                            nc.vector.tensor_add(
                                osb[:],
                                stt["pout"][:],
                                bob_t[:, nh * NQ : (nh + 1) * NQ],
                            )
                            nc.sync.dma_start(
                                out[ts, nh * NQ : (nh + 1) * NQ], osb[:]
                            )

                        units.append(ua)
                        units.append(ub)
                return units

            def attention(pr, st, next_units, pump_rate, norm_ref, carry_ref,
                          tail_units=None):
                """Attention for pair pr using st['qt'/'kt'/'vn*'], pumping
                next pair's units between chunk iterations.  Both heads'
                scores issue back-to-back as concurrent 64x128 PE row-tiles
                into adjacent PSUM banks; one exp covers both.  Two PV
                accumulators run per q-block (head 1's V tile is column-
                shifted so its output lands on partitions 64..127).  The
                last PV + eviction of each q-block are carried into the
                NEXT q-block's first chunk so the PE never drains waiting
                on the final exp; normalization is deferred likewise."""
                for qb in range(TQ // NQ):
                    qs = slice(qb * NQ, (qb + 1) * NQ)
                    qst = {}
                    for ch in range(T // P):
                        ps2 = ps_sc.tile([P, 2 * NQ], F32, tag="sc", name="ps2")
                        for hh in range(2):
                            h0 = hh * HD
                            nc.tensor.matmul(
                                ps2[:, hh * NQ : (hh + 1) * NQ],
                                st["kt"][h0 : h0 + HD, ch * P : (ch + 1) * P],
                                st["qt"][h0 : h0 + HD, qs],
                                start=True,
                                stop=True,
                            )
                        pexp = exp_p.tile(
                            [P, 2 * NQ], dt_mm, tag="ex", name="pexp"
                        )
                        nc.scalar.activation(
                            pexp[:],
                            ps2[:],
                            mybir.ActivationFunctionType.Exp,
                            scale=float(SCALE),
                        )
                        pump(next_units, pump_rate)
                        # tail out-projection blocks ride pair 7's loop;
                        # only after this pair's qb0 normalize is emitted
                        if tail_units and qb == 1 and ch >= 3:
                            pump(tail_units, 2)
                        # carried work (previous chunk's PV, or the prior
                        # q-block's last PV + eviction) runs one step behind
                        # so exp always has a full iteration of latency
                        if carry_ref[0] is not None:
                            carry_ref[0]()
                            carry_ref[0] = None
                        if ch == 2 and norm_ref[0] is not None:
                            norm_ref[0]()
                            norm_ref[0] = None

                        def make_pv(pexp=pexp, ch=ch, qst=qst, st=st):
                            def go():
                                if ch == 0:
                                    qst["po0"] = ps_po.tile(
                                        [P, NQ], F32, tag="po0", name="po0"
                                    )
                                    qst["po1"] = ps_po.tile(
                                        [P, NQ], F32, tag="po1", name="po1"
                                    )
                                nc.tensor.matmul(
                                    qst["po0"][:],
                                    st["vn0"][:, ch, :],
                                    pexp[:, 0:NQ],
                                    start=(ch == 0),
                                    stop=(ch == T // P - 1),
                                )
                                nc.tensor.matmul(
                                    qst["po1"][:],
                                    st["vn1"][:, ch, :],
                                    pexp[:, NQ : 2 * NQ],
                                    start=(ch == 0),
                                    stop=(ch == T // P - 1),
                                )

                            return go

                        carry_ref[0] = make_pv()

                    def wrap_qb_end(pv=carry_ref[0], qst=qst, qs=qs, pr=pr):
                        def go():
                            pv()
                            po0, po1 = qst["po0"], qst["po1"]
                            # evict unnormalized attn for both heads into
                            # one tile: rows 0:64 head0, rows 64:128 head1
                            au = smallp.tile([P, NQ], dt_mm, tag="au", name="au")
                            nc.vector.tensor_copy(au[0:HD, :], po0[0:HD, :])
                            nc.vector.tensor_copy(au[HD:, :], po1[HD:, :])
                            # denominators: head0 on partition 64 of po0,
                            # head1 on partition 0 of po1; f32r so they can
                            # feed the K=1 broadcast matmul directly
                            sums = sumsp.tile(
                                [1, 2 * NQ], mybir.dt.float32r,
                                tag="sums", name="sums",
                            )
                            with nc.allow_low_precision(
                                reason="denoms feed f32r bcast"
                            ):
                                nc.vector.tensor_copy(
                                    sums[:, 0:NQ], po0[HD : HD + 1, :]
                                )
                                nc.vector.tensor_copy(
                                    sums[:, NQ : 2 * NQ], po1[0:1, :]
                                )
                            norm_ref[0] = make_norm(pr, qs, au, sums)

                        return go

                    carry_ref[0] = wrap_qb_end()

            st, units = stage_units(0)
            pump(units, len(units))
            norm_ref = [None]
            carry_ref = [None]
            for pr in range(NPAIR):
                if pr + 1 < NPAIR:
                    nxt_st, nxt_units = stage_units(pr + 1)
                    tail_units = None
                else:
                    nxt_st, nxt_units = None, []
                    tail_units = make_tail_units()
                pump_rate = (len(nxt_units) + 31) // 32 if nxt_units else 0
                attention(pr, st, nxt_units, pump_rate, norm_ref, carry_ref,
                          tail_units)
                pump(nxt_units, len(nxt_units))
                if tail_units:
                    pump(tail_units, len(tail_units))
                st = nxt_st
            # flush pair 7 qb1's carried PV + eviction, then its normalize
            # (overlaps the out-projection tail; attn[7] is only consumed
            # by each block's final accumulation step)
            carry_ref[0]()
            carry_ref[0] = None
            norm_ref[0]()

        # ---- output projection (separate pool scope) ----
        with contextlib.ExitStack() as ctx:
            outp = ctx.enter_context(tc.tile_pool(name="outp", bufs=3))
            ps_o = ctx.enter_context(
                tc.tile_pool(name="ps_o", bufs=4, space="PSUM")
            )
            for tb in range(TQ // P // 2, TQ // P):
                ts = slice(tb * P, (tb + 1) * P)
                pout = [
                    ps_o.tile([P, NQ], F32, tag="pout", name="pout")
                    for _ in range(2)
                ]
                for ch in range(NPAIR):
                    for nh in range(2):
                        nc.tensor.matmul(
                            pout[nh][:],
                            attn[ch][:, ts],
                            wo_c[ch][:, nh * NQ : (nh + 1) * NQ],
                            start=(ch == 0),
                            stop=(ch == NPAIR - 1),
                        )
                osb = outp.tile([P, D], F32, tag="osb", name="osb")
                for nh in range(2):
                    nc.vector.tensor_add(
                        osb[:, nh * NQ : (nh + 1) * NQ],
                        pout[nh][:],
                        bob_t[:, nh * NQ : (nh + 1) * NQ],
                    )
                nc.sync.dma_start(out[ts, :], osb[:])

        persist_cm.__exit__(None, None, None)

    _split_sync_waits(nc)
    return nc


# ---------------- host-side input prep ----------------
def _np_dt(dt_mm):
    return ml_dtypes.bfloat16 if dt_mm == mybir.dt.bfloat16 else np.float32


def _cs_tiles(frac_b):
    """csa/csb [128, T] f32 RoPE tiles for one batch (frac_b: [T] f32)."""
    i = np.arange(HALF, dtype=np.float64)
    freq = (ROPE_BASE ** (2.0 * i / HD)).astype(np.float32)  # [32]
    pos = frac_b.astype(np.float32) * np.float32(ROPE_SCALE)
    ang = pos[None, :] / freq[:, None]  # [32, T] f32
    a64 = ang.astype(np.float64)
    cos = np.cos(a64).astype(np.float32)
    sin = np.sin(a64).astype(np.float32)
    csa = np.tile(cos, (4, 1))  # [128, T]
    csb = np.tile(np.concatenate([sin, -sin], axis=0), (2, 1))  # [128, T]
    return np.ascontiguousarray(csa), np.ascontiguousarray(csb)


def make_in_maps(x, frac, Wq, bq, Wk, bk, Wv, bv, Wo, bo, dt_mm=DT_MM):
    npdt = _np_dt(dt_mm)
    wqt = np.ascontiguousarray(Wq.T).astype(npdt)
    wkt = np.ascontiguousarray(Wk.T).astype(npdt)
    wvt = np.ascontiguousarray(Wv.T).astype(npdt)
    wot = np.ascontiguousarray(Wo.T).astype(npdt)
    bq_t = np.ascontiguousarray(bq.reshape(NPAIR, P).T).astype(np.float32)
    bk_t = np.ascontiguousarray(bk.reshape(NPAIR, P).T).astype(np.float32)
    bv_t = np.ascontiguousarray(bv.reshape(NPAIR, P).T).astype(np.float32)
    bob = np.ascontiguousarray(np.tile(bo[None, :], (P, 1))).astype(np.float32)
    in_maps = []
    for c in range(N_CORES):
        b, tqh = c // 2, c % 2
        xt = np.ascontiguousarray(x[b].T).astype(npdt)  # [D, T]
        xtq = np.ascontiguousarray(xt[:, tqh * TQ : (tqh + 1) * TQ])
        csa, csb = _cs_tiles(frac[b])
        in_maps.append(
            {
                "xt": xt,
                "xtq": xtq,
                "wqt": wqt,
                "wkt": wkt,
                "wvt": wvt,
                "wot": wot,
                "bq": bq_t,
                "bk": bk_t,
                "bv": bv_t,
                "bob": bob,
                "csaq": np.ascontiguousarray(
                    csa[:, tqh * TQ : (tqh + 1) * TQ]
                ).astype(npdt),
                "csbq": np.ascontiguousarray(
                    csb[:, tqh * TQ : (tqh + 1) * TQ]
                ).astype(npdt),
                "csak": csa.astype(npdt),
                "csbk": csb.astype(npdt),
            }
        )
    return in_maps


_NC_CACHE = {}


def _get_nc(dt_mm=DT_MM):
    key = str(dt_mm)
    if key not in _NC_CACHE:
        _NC_CACHE[key] = build_nc(dt_mm)
    return _NC_CACHE[key]


def kernel(x, frac, Wq, bq, Wk, bk, Wv, bv, Wo, bo):
    install_shims()
    from concourse.bass_utils import run_bass_kernel_spmd

    x = np.asarray(x, dtype=np.float32)
    frac = np.asarray(frac, dtype=np.float32)
    args = [np.asarray(a, dtype=np.float32) for a in (Wq, bq, Wk, bk, Wv, bv, Wo, bo)]
    in_maps = make_in_maps(x, frac, *args, dt_mm=DT_MM)
    nc = _get_nc(DT_MM)
    res = run_bass_kernel_spmd(nc, in_maps, list(range(N_CORES)))
    out = np.empty((B, T, D), dtype=np.float32)
    for c in range(N_CORES):
        b, tqh = c // 2, c % 2
        out[b, tqh * TQ : (tqh + 1) * TQ, :] = res.results[c]["out"]
    return out

